# revision 1
# baseline (speedup 1.0000x reference)
"""Banded (sparse) attention encoder block on 8 Trainium2 NeuronCores.

Problem: nn_NeuralEncoder (B=4, S=2000=100 time patches x 20 space patches,
H=512, 8 heads, D=64, RoPE over time-patch timestamps, banded attention
|t_q - t_k| <= 4 tiled over space patches).

Sharding: 8 cores = 4 batches x 2 head-groups (4 heads each).
Host prep: permute tokens to time-major order (u = t*SP + sp) so the banded
mask becomes a contiguous band of keys; transpose x to xT [H, S]; per-patch
RoPE cos/sin tables; slice weights per head-group.

Device (one SPMD Bass program, all matmuls bf16 with fp32 PSUM), fully
software-pipelined as ONE loop over the 20 key chunks so every engine and
the PE array stay dense (the TRN2 HAM clock-gate re-throttles the PE to
1.2 GHz whenever array activity dips, which was the baseline's main cost):
  - inputs stream in on two HWDGE rings (x column-block-major on the ACT
    ring, weights/tables on the Sync ring) so the first projection matmul
    issues ~10us in; per-patch RoPE tables are expanded to per-token on DVE
  - q/k projection calls (4 matmuls + PE rotate-half via a +-1 permutation
    matmul; cos/sin multiplies on DVE at 2x, add on GPSIMD) are spread one
    per chunk, one 500-column block ahead of the score chunks that need them
  - per chunk: v projection; scoresT [100 keys, <=280 queries] (exp on ACT,
    band-mask multiply split DVE/GPSIMD); AV strip [q, head, d+1]
    PSUM-accumulated over the 3 contributing chunks with the attn-mask ones
    column as softmax denominator; per-partition reciprocal + one
    broadcast-AP multiply normalizes; PE transposes rebuild ctxT [hd, s]
  - the Wo output projection + output DMA (bf16) interleave one 128-row
    column block per chunk as soon as their ctxT strips complete
Host epilogue: sum the two head-group partials per batch, add bo, transpose,
un-permute back to the original space-major token order.
"""

import numpy as np
import ml_dtypes
from contextlib import ExitStack

import concourse.tile as tile
from concourse import bacc, mybir
from concourse import bass_utils

F32 = mybir.dt.float32
BF16 = mybir.dt.bfloat16

# Static problem configuration (hardcoded, matches the reference).
B, T, SP = 4, 100, 20
S = T * SP                  # 2000
H, NH, D = 512, 8, 64
CF = CB = 4
G = 2                       # head groups (tensor-parallel factor)
HPC = NH // G               # heads per core = 4
HG = HPC * D                # 256 hidden per group
VW = HPC * (D + 1)          # 260: v layout with denominator column per head
VP = 95                     # padded per-head v width: AV matmuls stream 95
                            # cols (vs 65) to keep PE array duty above the
                            # HAM clock-gate threshold; pad cols are unread
ROPE_BASE = 10000.0
N_CORES = 8

PPC = 5                     # time patches per key chunk
CK = PPC * SP               # 100 keys per chunk
NCH = T // PPC              # 20 key chunks / query strips
SC = 500                    # free-dim chunk for [128, 500] psum tiles
NSC = S // SC               # 4
NPB = SC // SP              # 25 patches per 500-col block
MW = 280                    # max scoresT query-window width

_CACHE = {}


def _qwin(j):
    """Token range of the query window covered by scoresT of key chunk j."""
    lo = max(0, PPC * j - PPC) * SP
    hi = min(T, PPC * j + PPC + CF) * SP
    return lo, hi


def _build_program():
    nc = bacc.Bacc("TRN2", target_bir_lowering=False, debug=False,
                   num_devices=N_CORES)

    xT = nc.dram_tensor("xT", [H, S], BF16, kind="ExternalInput").ap()
    wq = nc.dram_tensor("wq", [H, HG], BF16, kind="ExternalInput").ap()
    wk = nc.dram_tensor("wk", [H, HG], BF16, kind="ExternalInput").ap()
    wv = nc.dram_tensor("wv", [H, VW], BF16, kind="ExternalInput").ap()
    wo = nc.dram_tensor("wo", [HG, H], BF16, kind="ExternalInput").ap()
    cosT = nc.dram_tensor("cosT", [128, T], BF16, kind="ExternalInput").ap()
    sinT = nc.dram_tensor("sinT", [128, T], BF16, kind="ExternalInput").ap()
    p128 = nc.dram_tensor("p128", [128, 128], BF16, kind="ExternalInput").ap()
    ident = nc.dram_tensor("ident", [CK, CK], BF16, kind="ExternalInput").ap()
    m_int = nc.dram_tensor("m_int", [CK, MW], BF16, kind="ExternalInput").ap()
    m_first = nc.dram_tensor("m_first", [CK, 180], BF16,
                             kind="ExternalInput").ap()
    am = nc.dram_tensor("am", [NCH, CK, HPC], F32, kind="ExternalInput").ap()
    outT = nc.dram_tensor("outT", [H, S], BF16, kind="ExternalOutput").ap()

    with ExitStack() as ctx:
        tc = ctx.enter_context(tile.TileContext(nc))
        consts = ctx.enter_context(tc.tile_pool(name="consts", bufs=1))
        persist = consts
        work = ctx.enter_context(tc.tile_pool(name="work", bufs=48))
        epool = work
        psum = ctx.enter_context(tc.tile_pool(name="psum", bufs=6,
                                              space="PSUM"))

        # ---- constants into SBUF (ordered so the first q-projection matmul
        # can start as soon as wq + xt chunk 0 land) ----
        wq_sb = consts.tile([128, 4, HG], BF16, tag="wq")
        for kc in range(4):
            nc.sync.dma_start(out=wq_sb[:, kc, :],
                              in_=wq[128 * kc:128 * (kc + 1), :])
        p_sb = consts.tile([128, 128], BF16, tag="p128")
        nc.sync.dma_start(out=p_sb, in_=p128)
        # x arrives column-block-major on the ACT HWDGE ring (parallel with
        # the Sync ring) so the first projection matmuls start early
        xt = [consts.tile([128, S], BF16, tag=f"xt{kc}", name=f"xt{kc}")
              for kc in range(4)]
        for sc in range(NSC):
            for kc in range(4):
                nc.scalar.dma_start(
                    out=xt[kc][:, SC * sc:SC * (sc + 1)],
                    in_=xT[128 * kc:128 * (kc + 1), SC * sc:SC * (sc + 1)])
        wk_sb = consts.tile([128, 4, HG], BF16, tag="wk")
        for kc in range(4):
            nc.sync.dma_start(out=wk_sb[:, kc, :],
                              in_=wk[128 * kc:128 * (kc + 1), :])
        cos_sb = consts.tile([128, T], BF16, tag="cos")
        nc.sync.dma_start(out=cos_sb, in_=cosT)
        sin_sb = consts.tile([128, T], BF16, tag="sin")
        nc.sync.dma_start(out=sin_sb, in_=sinT)
        # expand per-patch RoPE tables to per-token on DVE (2x_2p copy; DVE
        # is idle during the input DMA) so the RoPE multiplies run at 2x
        cosF = persist.tile([128, S], BF16, tag="cosF", name="cosF")
        sinF = persist.tile([128, S], BF16, tag="sinF", name="sinF")
        nc.vector.tensor_copy(
            out=cosF.rearrange("p (t s) -> p t s", s=SP),
            in_=cos_sb.unsqueeze(2).broadcast_to([128, T, SP]))
        nc.vector.tensor_copy(
            out=sinF.rearrange("p (t s) -> p t s", s=SP),
            in_=sin_sb.unsqueeze(2).broadcast_to([128, T, SP]))
        mf_sb = consts.tile([CK, 180], BF16, tag="mf")
        nc.sync.dma_start(out=mf_sb, in_=m_first)
        mi_sb = consts.tile([CK, MW], BF16, tag="mi")
        nc.sync.dma_start(out=mi_sb, in_=m_int)
        am_sb = consts.tile([CK, NCH, HPC], F32, tag="am")
        nc.sync.dma_start(out=am_sb, in_=am.rearrange("c p f -> p c f"))

        wv_sb = consts.tile([128, 4, VW], BF16, tag="wv")
        nc.sync.dma_start(out=wv_sb, in_=wv.rearrange("(c p) m -> p c m", p=128))
        id_sb = consts.tile([CK, CK], BF16, tag="ident")
        nc.sync.dma_start(out=id_sb, in_=ident)
        wo_sb = consts.tile([128, 2, H], BF16, tag="wo")
        nc.sync.dma_start(out=wo_sb, in_=wo.rearrange("(c p) m -> p c m", p=128))

        # ---- persistent activations ----
        qT = [persist.tile([128, S], BF16, tag=f"qT{hp}", name=f"qT{hp}")
              for hp in range(2)]
        kT = [persist.tile([128, S], BF16, tag=f"kT{hp}", name=f"kT{hp}")
              for hp in range(2)]
        ctxT = [persist.tile([128, S], BF16, tag=f"ctxT{hp}", name=f"ctxT{hp}")
                for hp in range(2)]
        v_sb = [persist.tile([CK, HPC, VP], BF16, tag=f"v{vc}",
                             name=f"v{vc}")
                for vc in range(NCH)]

        # ---- q/k projections + RoPE (rotate-half via PE matmul) ----
        def qk_proj(w_sb, dst, hp, sc):
            cols = slice(SC * sc, SC * (sc + 1))
            ps = psum.tile([128, SC], F32, tag="pp", bufs=2)
            for kc in range(4):
                nc.tensor.matmul(
                    ps,
                    lhsT=w_sb[:, kc, 128 * hp:128 * (hp + 1)],
                    rhs=xt[kc][:, cols],
                    start=(kc == 0), stop=(kc == 3),
                )
            pre = work.tile([128, SC], BF16, tag="pre", bufs=3)
            nc.scalar.copy(out=pre, in_=ps)
            psr = psum.tile([128, SC], F32, tag="pp", bufs=2)
            nc.tensor.matmul(psr, lhsT=p_sb, rhs=pre, start=True, stop=True)
            t1 = work.tile([128, SC], BF16, tag="t1", bufs=3)
            nc.vector.tensor_mul(out=t1, in0=pre, in1=cosF[:, cols])
            t2 = work.tile([128, SC], BF16, tag="t2", bufs=3)
            nc.vector.tensor_mul(out=t2, in0=psr, in1=sinF[:, cols])
            nc.gpsimd.tensor_add(out=dst[:, cols], in0=t1, in1=t2)

        # ---- v projection (natural layout, 65-col stride per head) ----
        def v_proj(vc):
            rows = slice(CK * vc, CK * (vc + 1))
            ps = psum.tile([CK, VW], F32, tag="pv", bufs=1)
            for kc in range(4):
                nc.tensor.matmul(
                    ps,
                    lhsT=xt[kc][:, rows],
                    rhs=wv_sb[:, kc, :],
                    start=(kc == 0), stop=(kc == 3),
                )
            # scale rows by attn_mask (ones in practice), cast to bf16
            nc.vector.tensor_scalar_mul(
                v_sb[vc][:, :, 0:D + 1],
                ps.rearrange("p (h e) -> p h e", e=D + 1),
                am_sb[:, vc, 0:1])
            # write denominator column (attn_mask value) per head
            nc.vector.tensor_copy(out=v_sb[vc][:, :, D], in_=am_sb[:, vc, :])

        # ---- attention ----
        exp_t = {}
        cs_t = {}

        def scores_chunk(j):
            qlo, qhi = _qwin(j)
            w = qhi - qlo
            for h in range(HPC):
                hp, hb = h // 2, 64 * (h % 2)
                ps = psum.tile([CK, MW], F32, tag="pss", bufs=3)
                nc.tensor.matmul(
                    ps[:, :w],
                    lhsT=kT[hp][hb:hb + 64, CK * j:CK * (j + 1)],
                    rhs=qT[hp][hb:hb + 64, qlo:qhi],
                    start=True, stop=True,
                )
                et = epool.tile([CK, MW], BF16, tag="exp", bufs=16)
                nc.scalar.activation(out=et[:, :w], in_=ps[:, :w],
                                     func=mybir.ActivationFunctionType.Exp,
                                     scale=0.125)
                mask = mf_sb if j == 0 else mi_sb[:, :w]
                # band-mask multiply: split across DVE / GPSIMD
                eng = nc.vector if h < 2 else nc.gpsimd
                eng.tensor_mul(out=et[:, :w], in0=et[:, :w], in1=mask)
                exp_t[(j, h)] = et

        def av_mm(i):
            # chunk i first: it covers the strip fully (start=True sets
            # has_written; the left neighbor accumulates on partitions 0:80)
            chunks = [c for c in (i, i - 1, i + 1) if 0 <= c < NCH]
            ps = psum.tile([CK, HPC, VP], F32, tag="pav", bufs=1)
            for h in range(HPC):
                for n, j in enumerate(chunks):
                    qlo, qhi = _qwin(j)
                    lo_g, hi_g = max(CK * i, qlo), min(CK * i + CK, qhi)
                    nc.tensor.matmul(
                        ps[0:hi_g - lo_g, h, :],
                        lhsT=exp_t[(j, h)][:, lo_g - qlo:hi_g - qlo],
                        rhs=v_sb[j][:, h, :],
                        start=(n == 0), stop=(n == len(chunks) - 1),
                    )
            # per-query softmax normalization: reciprocal of the denominator
            # column, one broadcast-AP multiply for all 4 heads
            rcp = work.tile([CK, HPC], F32, tag="rcp", bufs=3)
            nc.vector.reciprocal(out=rcp, in_=ps[:, :, D])
            cs = work.tile([CK, HPC, D], BF16, tag="cs", bufs=3)
            nc.vector.tensor_mul(
                out=cs, in0=ps[:, :, 0:D],
                in1=rcp.unsqueeze(2).broadcast_to([CK, HPC, D]))
            cs_t[i] = cs

        def av_tr(i):
            # transpose [100 q, 128 hd-pair] -> ctxT [128, 100] per pair
            csf = cs_t.pop(i).rearrange("p h e -> p (h e)")
            for hp in range(2):
                pt = psum.tile([128, CK], BF16, tag="ptr", bufs=1)
                nc.tensor.transpose(pt, csf[:, 128 * hp:128 * (hp + 1)], id_sb)
                nc.vector.tensor_copy(out=ctxT[hp][:, CK * i:CK * (i + 1)],
                                      in_=pt)

        # ---- output projection, one 128-row column block at a time ----
        def out_oc(c, oc, lo=0, hi=SC):
            w = hi - lo
            cols = slice(SC * c + lo, SC * c + hi)
            ps = psum.tile([128, SC], F32, tag="pp", bufs=2)
            for hp in range(2):
                nc.tensor.matmul(
                    ps[:, :w],
                    lhsT=wo_sb[:, hp, 128 * oc:128 * (oc + 1)],
                    rhs=ctxT[hp][:, cols],
                    start=(hp == 0), stop=(hp == 1),
                )
            ost = work.tile([128, SC], BF16, tag="ost", bufs=3)
            if oc % 2 == 0:
                nc.scalar.copy(out=ost[:, :w], in_=ps[:, :w])
            else:
                nc.vector.tensor_copy(out=ost[:, :w], in_=ps[:, :w])
            nc.sync.dma_start(out=outT[128 * oc:128 * (oc + 1), cols],
                              in_=ost[:, :w])

        # ---- software-pipelined main loop ----
        # q/k projection block sc feeds score chunks [ranges[sc], ranges[sc+1])
        # (chunk j's query window ends at patch 5j+9 <= 25(sc+1)); the next
        # group's 4 projection calls are spread one per chunk so the PE gets
        # a uniform stream of dense N=500 matmuls (keeps the HAM un-throttled)
        def qk_call(sc, m):
            hp = m // 2
            if m % 2 == 0:
                qk_proj(wq_sb, qT[hp], hp, sc)
            else:
                qk_proj(wk_sb, kT[hp], hp, sc)

        ranges = [0, 4, 9, 14, NCH]
        for m in range(4):
            qk_call(0, m)
        for sc in range(NSC):
            for idx, j in enumerate(range(ranges[sc], ranges[sc + 1])):
                if sc + 1 < NSC and idx < 4:
                    qk_call(sc + 1, idx)
                v_proj(j)
                scores_chunk(j)
                if j >= 2:
                    av_tr(j - 2)
                if j >= 1:
                    av_mm(j - 1)
                # spread output projection: block c ready after av_tr(5c+4)
                if j >= 6 and (j - 6) % 5 < 4 and (j - 6) // 5 < 3:
                    out_oc((j - 6) // 5, (j - 6) % 5)
        # epilogue: drain the pipeline; the last output block is split so
        # its first 400 columns (strips 15-18) overlap the final strip's work
        av_mm(NCH - 1)
        av_tr(NCH - 2)
        for oc in range(4):
            out_oc(3, oc, 0, 400)
        av_tr(NCH - 1)
        for oc in range(4):
            out_oc(3, oc, 400, SC)

    nc.finalize()   # Bacc register allocation + DCE before serialization
    return nc


def _get_program():
    if "nc" not in _CACHE:
        _CACHE["nc"] = _build_program()
    return _CACHE["nc"]


def _host_prep(x, attn_mask, timestamps, Wq, Wk, Wv, Wo):
    """Build the 8 per-core input maps."""
    bf16 = ml_dtypes.bfloat16

    def to_tm(a):
        # [B, S, ...] space-major -> time-major (u = t*SP + sp)
        return (a.reshape(B, SP, T, *a.shape[2:])
                 .swapaxes(1, 2)
                 .reshape(B, S, *a.shape[2:]))

    x_tm = to_tm(np.ascontiguousarray(x))
    ts_tm = to_tm(np.ascontiguousarray(timestamps))
    amask_tm = to_tm(np.ascontiguousarray(attn_mask)).astype(np.float32)

    # the device program bakes the time-patch structure into its band masks
    # and per-patch RoPE tables; the reference generates exactly this pattern
    assert np.array_equal(
        ts_tm, np.broadcast_to(np.repeat(np.arange(T, dtype=ts_tm.dtype), SP),
                               (B, S))), "unexpected timestamp pattern"

    # RoPE tables, per time patch (expanded to per-token on device)
    inv_freq = 1.0 / (ROPE_BASE ** (np.arange(0, D, 2, dtype=np.float32) / D))
    tt = np.arange(T, dtype=np.float32)
    freqs = tt[:, None] * inv_freq[None, :]
    emb = np.concatenate([freqs, freqs], axis=-1)      # [T, D]
    cos_t = np.cos(emb).astype(np.float32).T           # [64, T]
    sin_t = np.sin(emb).astype(np.float32).T

    # rotation matrix (sign-carrying rotate-half), block-diag per head pair
    p = np.zeros((128, 128), np.float32)
    for blk in (0, 64):
        for d in range(32):
            p[blk + d + 32, blk + d] = -1.0
            p[blk + d, blk + d + 32] = 1.0

    # band masks (interior window starts at patch 5j-5; first at 0)
    kr = np.arange(CK)[:, None] // SP          # key patch within chunk [0,5)
    dlt = np.arange(MW)[None, :] // SP - kr
    m_int = ((dlt >= 1) & (dlt <= 9)).astype(np.float32)
    dlt0 = np.arange(180)[None, :] // SP - kr
    m_first = ((dlt0 >= -4) & (dlt0 <= 4)).astype(np.float32)

    in_maps = []
    for c in range(N_CORES):
        b, g = c // 2, c % 2
        hcols = slice(HG * g, HG * (g + 1))

        wv_ext = np.zeros((H, VW), np.float32)
        for h in range(HPC):
            wv_ext[:, (D + 1) * h:(D + 1) * h + D] = \
                Wv[:, HG * g + D * h:HG * g + D * (h + 1)]

        in_maps.append({
            "xT": np.ascontiguousarray(x_tm[b].T).astype(bf16),
            "wq": np.ascontiguousarray(Wq[:, hcols]).astype(bf16),
            "wk": np.ascontiguousarray(Wk[:, hcols]).astype(bf16),
            "wv": wv_ext.astype(bf16),
            "wo": np.ascontiguousarray(Wo[hcols, :]).astype(bf16),
            "cosT": np.vstack([cos_t, cos_t]).astype(bf16),
            "sinT": np.vstack([sin_t, sin_t]).astype(bf16),
            "p128": p.astype(bf16),
            "ident": np.eye(CK, dtype=np.float32).astype(bf16),
            "m_int": m_int.astype(bf16),
            "m_first": m_first.astype(bf16),
            "am": np.ascontiguousarray(
                np.repeat(amask_tm[b].reshape(NCH, CK, 1), HPC, axis=2)),
        })
    return in_maps


def kernel(x, attn_mask, timestamps, Wq, bq, Wk, bk, Wv, bv, Wo, bo,
           **_ignored):
    x = np.asarray(x, np.float32)
    attn_mask = np.asarray(attn_mask)
    timestamps = np.asarray(timestamps)
    Wq, Wk, Wv, Wo = (np.asarray(a, np.float32) for a in (Wq, Wk, Wv, Wo))
    bq, bk, bv, bo = (np.asarray(a, np.float32) for a in (bq, bk, bv, bo))
    assert not (np.any(bq) or np.any(bk) or np.any(bv)), \
        "nonzero qkv biases not supported"

    nc = _get_program()
    in_maps = _host_prep(x, attn_mask, timestamps, Wq, Wk, Wv, Wo)

    res = bass_utils.run_bass_kernel_spmd(nc, in_maps,
                                          core_ids=list(range(N_CORES)))
    _CACHE["last_results"] = res

    out = np.empty((B, S, H), np.float32)
    for b in range(B):
        o = (res.results[2 * b]["outT"].astype(np.float32) +
             res.results[2 * b + 1]["outT"].astype(np.float32))
        o_tm = o.T + bo[None, :]                        # [2000, 512]
        out[b] = (o_tm.reshape(T, SP, H)
                      .swapaxes(0, 1)
                      .reshape(S, H))
    return out



# revision 11
# speedup vs baseline: 1.1249x; 1.1249x over previous
"""Banded (sparse) attention encoder block on 8 Trainium2 NeuronCores.

Problem: nn_NeuralEncoder (B=4, S=2000=100 time patches x 20 space patches,
H=512, 8 heads, D=64, RoPE over time-patch timestamps, banded attention
|t_q - t_k| <= 4 tiled over space patches).

Sharding: 8 cores = 4 batches x 2 head-groups (4 heads each).
Host prep: permute tokens to time-major order (u = t*SP + sp) so the banded
mask becomes a contiguous band of keys; transpose x to xT [H, S]; weights
pre-packed so every input DMA is fully contiguous; RoPE sin table carries the
rotate-half signs.

Device (one SPMD Bass program, all matmuls bf16 with fp32 PSUM):
  - inputs stream on both HWDGE rings, first-needed first; late x columns are
    fetched from inside the main loop so the rings never block early compute
  - q/k projection: 4 matmuls -> psum; DVE multiplies psum by per-patch
    cos/sin tables (broadcast APs, no expansion pass); rotate-half happens as
    two SWDGE accumulate-DMAs (partition block-swap, sign baked into sinS2)
  - attn_mask==1 fast path: no mask scaling anywhere; the softmax denominator
    column of v is a one-time memset
  - per chunk j (100 keys): v projection (psum -> bf16 cast on gpsimd);
    scoresT [100 keys, <=280 queries] per head -> exp on ACT into one fused
    [100, 4 heads, 280] tile; band masking only multiplies the two staircase
    side-blocks (cols 0:100 / 200:280) - the 100-col center is always valid
  - AV strip [q, head, d] accumulated over <=3 chunks with a ones column as
    denominator; reciprocal+normalize; PE transposes rebuild ctxT; output
    projection + DMA interleaved one 128-row column block per chunk
Host epilogue: sum the two head-group partials, add bo, transpose, un-permute
back to space-major order. Falls back to a numpy reference path if the inputs
don't match the expected mask/timestamp structure.
"""

import numpy as np
import ml_dtypes
from contextlib import ExitStack

import concourse.tile as tile
from concourse import bacc, mybir
from concourse import bass_utils

F32 = mybir.dt.float32
BF16 = mybir.dt.bfloat16

# Static problem configuration (hardcoded, matches the reference).
B, T, SP = 4, 100, 20
S = T * SP                  # 2000
H, NH, D = 512, 8, 64
CF = CB = 4
G = 2                       # head groups (tensor-parallel factor)
HPC = NH // G               # heads per core = 4
HG = HPC * D                # 256 hidden per group
VP = 95                     # padded per-head v width (pad cols keep the PE
                            # array duty above the HAM clock-gate threshold)
ROPE_BASE = 10000.0
N_CORES = 8

PPC = 5                     # time patches per key chunk
CK = PPC * SP               # 100 keys per chunk
NCH = T // PPC              # 20 key chunks / query strips
SC = 500                    # free-dim chunk for [128, 500] psum tiles
NSC = S // SC               # 4
MW = 280                    # max scoresT query-window width

ROT = "pe"                  # rotate-half impl: dmaacc | dma | pe

_CACHE = {}


def _qwin(j):
    """Token range of the query window covered by scoresT of key chunk j."""
    lo = max(0, PPC * j - PPC) * SP
    hi = min(T, PPC * j + PPC + CF) * SP
    return lo, hi


def _build_program():
    nc = bacc.Bacc("TRN2", target_bir_lowering=False, debug=False,
                   num_devices=N_CORES)

    xT = nc.dram_tensor("xT", [H, S], BF16, kind="ExternalInput").ap()
    wq = nc.dram_tensor("wq", [128, 4, HG], BF16, kind="ExternalInput").ap()
    wk = nc.dram_tensor("wk", [128, 4, HG], BF16, kind="ExternalInput").ap()
    wv = nc.dram_tensor("wv", [128, 4, HG], BF16, kind="ExternalInput").ap()
    wo = nc.dram_tensor("wo", [128, 2, H], BF16, kind="ExternalInput").ap()
    cosT = nc.dram_tensor("cosT", [128, T], BF16, kind="ExternalInput").ap()
    sinS = nc.dram_tensor("sinS", [128, T], BF16, kind="ExternalInput").ap()
    m2L = nc.dram_tensor("m2L", [CK, CK], BF16, kind="ExternalInput").ap()
    m2R = nc.dram_tensor("m2R", [CK, 80], BF16, kind="ExternalInput").ap()
    ident = nc.dram_tensor("ident", [CK, CK], BF16, kind="ExternalInput").ap()
    p128 = nc.dram_tensor("p128", [128, 128], BF16, kind="ExternalInput").ap()
    outT = nc.dram_tensor("outT", [H, S], BF16, kind="ExternalOutput").ap()

    with ExitStack() as ctx:
        tc = ctx.enter_context(tile.TileContext(nc))
        consts = ctx.enter_context(tc.tile_pool(name="consts", bufs=1))
        work = ctx.enter_context(tc.tile_pool(name="work", bufs=48))
        psum = ctx.enter_context(tc.tile_pool(name="psum", bufs=8,
                                              space="PSUM"))

        # ---- persistent tiles ----
        xt = [consts.tile([128, S], BF16, tag=f"xt{kc}", name=f"xt{kc}")
              for kc in range(4)]
        wq_sb = consts.tile([128, 4, HG], BF16, tag="wq")
        wk_sb = consts.tile([128, 4, HG], BF16, tag="wk")
        wv_sb = consts.tile([128, 4, HG], BF16, tag="wv")
        wo_sb = consts.tile([128, 2, H], BF16, tag="wo")
        cos_sb = consts.tile([128, T], BF16, tag="cos")
        sin_sb = consts.tile([128, T], BF16, tag="sin")
        mL_sb = consts.tile([CK, CK], BF16, tag="mL")
        mR_sb = consts.tile([CK, 80], BF16, tag="mR")
        id_sb = consts.tile([CK, CK], BF16, tag="ident")
        if ROT == "pe":
            p_sb = consts.tile([128, 128], BF16, tag="p128")

        qT = [consts.tile([128, S], BF16, tag=f"qT{hp}", name=f"qT{hp}")
              for hp in range(2)]
        kT = [consts.tile([128, S], BF16, tag=f"kT{hp}", name=f"kT{hp}")
              for hp in range(2)]
        ctx_all = consts.tile([128, 2, S], BF16, tag="ctx", name="ctx")
        v_all = consts.tile([CK, NCH, HPC, VP], BF16, tag="v", name="v")

        # ---- input DMAs: first-needed first, split across both rings ----
        # sync ring
        nc.sync.dma_start(out=wq_sb, in_=wq)
        nc.sync.dma_start(out=cos_sb, in_=cosT)
        nc.sync.dma_start(out=sin_sb, in_=sinS)
        nc.sync.dma_start(out=xt[0][:, 0:SC], in_=xT[0:128, 0:SC])
        nc.sync.dma_start(out=xt[1][:, 0:SC], in_=xT[128:256, 0:SC])
        nc.sync.dma_start(out=wk_sb, in_=wk)
        nc.sync.dma_start(out=mL_sb, in_=m2L)
        nc.sync.dma_start(out=mR_sb, in_=m2R)
        nc.sync.dma_start(out=xt[0][:, SC:1000], in_=xT[0:128, SC:1000])
        nc.sync.dma_start(out=xt[1][:, SC:1000], in_=xT[128:256, SC:1000])
        nc.sync.dma_start(out=wv_sb, in_=wv)
        nc.sync.dma_start(out=id_sb, in_=ident)
        nc.sync.dma_start(out=wo_sb, in_=wo)
        if ROT == "pe":
            nc.sync.dma_start(out=p_sb, in_=p128)
        # scalar ring
        nc.scalar.dma_start(out=xt[2][:, 0:SC], in_=xT[256:384, 0:SC])
        nc.scalar.dma_start(out=xt[3][:, 0:SC], in_=xT[384:512, 0:SC])
        nc.scalar.dma_start(out=xt[2][:, SC:1000], in_=xT[256:384, SC:1000])
        nc.scalar.dma_start(out=xt[3][:, SC:1000], in_=xT[384:512, SC:1000])

        # softmax denominator column of v (attn_mask == 1)
        nc.gpsimd.memset(v_all[:, :, :, D], 1.0)

        # ---- q/k projections + RoPE ----
        def qk_proj(w_sb, dst, hp, sc):
            cols = slice(SC * sc, SC * (sc + 1))
            p0 = 25 * sc
            ps = psum.tile([128, SC], F32, tag="pp", bufs=2)
            for kc in range(4):
                nc.tensor.matmul(
                    ps,
                    lhsT=w_sb[:, kc, 128 * hp:128 * (hp + 1)],
                    rhs=xt[kc][:, cols],
                    start=(kc == 0), stop=(kc == 3),
                )
            psv = ps.rearrange("p (a b) -> p a b", b=SP)
            # t1 = ps * cos  -> straight into dst
            nc.vector.tensor_mul(
                out=dst[:, cols].rearrange("p (a b) -> p a b", b=SP),
                in0=psv,
                in1=cos_sb[:, p0:p0 + 25].unsqueeze(2)
                    .broadcast_to([128, 25, SP]))
            # t2 = ps * (sign-folded sin); rotate-half accumulated into dst
            t2p = work.tile([128, SC], BF16, tag="t2p", bufs=3)
            nc.vector.tensor_mul(
                out=t2p.rearrange("p (a b) -> p a b", b=SP),
                in0=psv,
                in1=sin_sb[:, p0:p0 + 25].unsqueeze(2)
                    .broadcast_to([128, 25, SP]))
            if ROT == "dmaacc":
                add = mybir.AluOpType.add
                sv = t2p.rearrange("(b h p) f -> b h p f", b=2, h=2)
                dv = dst[:, cols].rearrange("(b h p) f -> b h p f", b=2, h=2)
                nc.gpsimd.dma_start(out=dv[:, 0], in_=sv[:, 1], accum_op=add)
                nc.gpsimd.dma_start(out=dv[:, 1], in_=sv[:, 0], accum_op=add)
            elif ROT == "dma":
                prerot = work.tile([128, SC], BF16, tag="prerot", bufs=3)
                sv = t2p.rearrange("(b h p) f -> b h p f", b=2, h=2)
                dv = prerot.rearrange("(b h p) f -> b h p f", b=2, h=2)
                nc.sync.dma_start(out=dv[:, 0], in_=sv[:, 1])
                nc.sync.dma_start(out=dv[:, 1], in_=sv[:, 0])
                nc.gpsimd.tensor_add(out=dst[:, cols], in0=dst[:, cols],
                                     in1=prerot)
            else:  # pe
                psr = psum.tile([128, SC], F32, tag="pp", bufs=2)
                nc.tensor.matmul(psr, lhsT=p_sb, rhs=t2p, start=True,
                                 stop=True)
                nc.vector.tensor_add(out=dst[:, cols], in0=dst[:, cols],
                                     in1=psr)

        # ---- v projection (psum shares the [128, 500] "pp" ring) ----
        def v_proj(vc):
            rows = slice(CK * vc, CK * (vc + 1))
            ps = psum.tile([128, SC], F32, tag="pp", bufs=2)
            for kc in range(4):
                nc.tensor.matmul(
                    ps[0:CK, 0:HG],
                    lhsT=xt[kc][:, rows],
                    rhs=wv_sb[:, kc, :],
                    start=(kc == 0), stop=(kc == 3),
                )
            nc.scalar.copy(
                out=v_all[:, vc, :, 0:D],
                in_=ps[0:CK, 0:HG].rearrange("p (h e) -> p h e", e=D))

        # ---- attention ----
        exp_t = {}
        cs_t = {}

        def scores_chunk(j):
            qlo, qhi = _qwin(j)
            w = qhi - qlo
            et = work.tile([CK, HPC, MW], BF16, tag="et4", bufs=5)
            for hp in range(2):
                # two heads' scores into one bank-aligned psum pair; one
                # exp activation evacuates both
                ps = psum.tile([CK, 2, 512], F32, tag="pss", bufs=2)
                for hh in range(2):
                    hb = 64 * hh
                    nc.tensor.matmul(
                        ps[:, hh, :w],
                        lhsT=kT[hp][hb:hb + 64, CK * j:CK * (j + 1)],
                        rhs=qT[hp][hb:hb + 64, qlo:qhi],
                        start=True, stop=True,
                    )
                nc.scalar.activation(out=et[:, 2 * hp:2 * hp + 2, :w],
                                     in_=ps[:, :, :w],
                                     func=mybir.ActivationFunctionType.Exp,
                                     scale=0.125)
            # band masking: only the staircase side-blocks need a multiply
            if j > 0:
                nc.vector.tensor_mul(
                    out=et[:, :, 0:CK], in0=et[:, :, 0:CK],
                    in1=mL_sb.unsqueeze(1).broadcast_to([CK, HPC, CK]))
            rlo = 100 if j == 0 else (None if j == NCH - 1 else 200)
            if rlo is not None:
                nc.gpsimd.tensor_mul(
                    out=et[:, :, rlo:rlo + 80], in0=et[:, :, rlo:rlo + 80],
                    in1=mR_sb.unsqueeze(1).broadcast_to([CK, HPC, 80]))
            exp_t[j] = et

        def av_mm(i):
            # chunk i first: it covers the strip fully (start=True sets
            # has_written; the left neighbor accumulates on partitions 0:80)
            chunks = [c for c in (i, i - 1, i + 1) if 0 <= c < NCH]
            ps = psum.tile([CK, HPC, VP], F32, tag="pav", bufs=1)
            for h in range(HPC):
                for n, j in enumerate(chunks):
                    qlo, qhi = _qwin(j)
                    lo_g, hi_g = max(CK * i, qlo), min(CK * i + CK, qhi)
                    nc.tensor.matmul(
                        ps[0:hi_g - lo_g, h, :],
                        lhsT=exp_t[j][:, h, lo_g - qlo:hi_g - qlo],
                        rhs=v_all[:, j, h, :],
                        start=(n == 0), stop=(n == len(chunks) - 1),
                    )
            rcp = work.tile([CK, HPC], F32, tag="rcp", bufs=3)
            nc.vector.reciprocal(out=rcp, in_=ps[:, :, D])
            cs = work.tile([CK, HPC, D], BF16, tag="cs", bufs=3)
            nc.vector.tensor_mul(
                out=cs, in0=ps[:, :, 0:D],
                in1=rcp.unsqueeze(2).broadcast_to([CK, HPC, D]))
            cs_t[i] = cs

        def av_tr(i):
            csf = cs_t.pop(i).rearrange("p h e -> p (h e)")
            pt = psum.tile([128, 2, CK], BF16, tag="ptr", bufs=1)
            for hp in range(2):
                nc.tensor.transpose(pt[:, hp, :],
                                    csf[:, 128 * hp:128 * (hp + 1)], id_sb)
            nc.vector.tensor_copy(out=ctx_all[:, :, CK * i:CK * (i + 1)],
                                  in_=pt)

        # ---- output projection, one 128-row column block at a time ----
        def out_oc(c, oc, lo=0, hi=SC):
            w = hi - lo
            cols = slice(SC * c + lo, SC * c + hi)
            ps = psum.tile([128, SC], F32, tag="pp", bufs=2)
            for hp in range(2):
                nc.tensor.matmul(
                    ps[:, :w],
                    lhsT=wo_sb[:, hp, 128 * oc:128 * (oc + 1)],
                    rhs=ctx_all[:, hp, cols],
                    start=(hp == 0), stop=(hp == 1),
                )
            ost = work.tile([128, SC], BF16, tag="ost", bufs=3)
            nc.scalar.copy(out=ost[:, :w], in_=ps[:, :w])
            nc.sync.dma_start(out=outT[128 * oc:128 * (oc + 1), cols],
                              in_=ost[:, :w])

        # ---- software-pipelined main loop ----
        def qk_call(sc, m):
            hp = m % 2
            if m < 2:
                qk_proj(wq_sb, qT[hp], hp, sc)
            else:
                qk_proj(wk_sb, kT[hp], hp, sc)

        ranges = [0, 4, 9, 14, NCH]
        for m in range(4):
            qk_call(0, m)
        for sc in range(NSC):
            for idx, j in enumerate(range(ranges[sc], ranges[sc + 1])):
                if sc + 1 < NSC and idx < 4:
                    qk_call(sc + 1, idx)
                if j == 2:
                    # late x columns, issued once the early DMAs have drained
                    nc.sync.dma_start(out=xt[0][:, 1000:S],
                                      in_=xT[0:128, 1000:S])
                    nc.scalar.dma_start(out=xt[2][:, 1000:S],
                                        in_=xT[256:384, 1000:S])
                if j == 3:
                    nc.sync.dma_start(out=xt[1][:, 1000:S],
                                      in_=xT[128:256, 1000:S])
                    nc.scalar.dma_start(out=xt[3][:, 1000:S],
                                        in_=xT[384:512, 1000:S])
                v_proj(j)
                scores_chunk(j)
                if j >= 2:
                    av_tr(j - 2)
                if j >= 1:
                    av_mm(j - 1)
                if j >= 6 and (j - 6) % 5 < 4 and (j - 6) // 5 < 3:
                    out_oc((j - 6) // 5, (j - 6) % 5)
        # epilogue: drain the pipeline; the last output block is split so
        # its first 400 columns (strips 15-18) overlap the final strip's work
        av_mm(NCH - 1)
        av_tr(NCH - 2)
        for oc in range(4):
            out_oc(3, oc, 0, 400)
        av_tr(NCH - 1)
        for oc in range(4):
            out_oc(3, oc, 400, SC)

    nc.finalize()   # Bacc register allocation + DCE before serialization
    return nc


def _get_program():
    if "nc" not in _CACHE:
        _CACHE["nc"] = _build_program()
    return _CACHE["nc"]


def _host_prep(x, Wq, Wk, Wv, Wo):
    """Build the 8 per-core input maps."""
    bf16 = ml_dtypes.bfloat16

    def to_tm(a):
        # [B, S, ...] space-major -> time-major (u = t*SP + sp)
        return (a.reshape(B, SP, T, *a.shape[2:])
                 .swapaxes(1, 2)
                 .reshape(B, S, *a.shape[2:]))

    x_tm = to_tm(np.ascontiguousarray(x))

    # RoPE tables, per time patch; sin carries the rotate-half signs
    inv_freq = 1.0 / (ROPE_BASE ** (np.arange(0, D, 2, dtype=np.float32) / D))
    tt = np.arange(T, dtype=np.float32)
    freqs = tt[:, None] * inv_freq[None, :]
    emb = np.concatenate([freqs, freqs], axis=-1)      # [T, D]
    cos_t = np.cos(emb).astype(np.float32).T           # [64, T]
    sin_t = np.sin(emb).astype(np.float32).T
    sinS = sin_t.copy()
    sinS[D // 2:] *= -1.0                              # sign fold for rotate

    # staircase band masks for the side-blocks of the score window
    kr = np.arange(CK)[:, None] // SP
    cl = np.arange(CK)[None, :] // SP
    m2L = (cl >= kr + 1).astype(np.float32)            # [100, 100]
    cr = np.arange(80)[None, :] // SP
    m2R = (cr <= kr - 1).astype(np.float32)            # [100, 80]

    # unsigned rotate-half permutation for the PE fallback (the sinS table
    # already carries the signs)
    p = np.zeros((128, 128), np.float32)
    for blk in (0, 64):
        for d in range(32):
            p[blk + d + 32, blk + d] = 1.0
            p[blk + d, blk + d + 32] = 1.0

    def pack4(w):       # [512, 256] -> [128, 4, 256]
        return np.ascontiguousarray(
            w.reshape(4, 128, w.shape[1]).transpose(1, 0, 2))

    in_maps = []
    for c in range(N_CORES):
        b, g = c // 2, c % 2
        hcols = slice(HG * g, HG * (g + 1))
        in_maps.append({
            "xT": np.ascontiguousarray(x_tm[b].T).astype(bf16),
            "wq": pack4(Wq[:, hcols]).astype(bf16),
            "wk": pack4(Wk[:, hcols]).astype(bf16),
            "wv": pack4(Wv[:, hcols]).astype(bf16),
            "wo": np.ascontiguousarray(
                Wo[hcols, :].reshape(2, 128, H).transpose(1, 0, 2)
            ).astype(bf16),
            "cosT": np.vstack([cos_t, cos_t]).astype(bf16),
            "sinS": np.vstack([sinS, sinS]).astype(bf16),
            "m2L": m2L.astype(bf16),
            "m2R": m2R.astype(bf16),
            "ident": np.eye(CK, dtype=np.float32).astype(bf16),
            "p128": p.astype(bf16),
        })
    return in_maps


def _numpy_fallback(x, attn_mask, timestamps, Wq, bq, Wk, bk, Wv, bv, Wo, bo):
    """Reference-equivalent numpy path for unexpected input structure."""
    inv_freq = 1.0 / (ROPE_BASE ** (np.arange(0, D, 2, dtype=np.float32) / D))
    tt = np.arange(T, dtype=np.float32)
    emb = np.concatenate([tt[:, None] * inv_freq[None, :]] * 2, axis=-1)
    cos_t, sin_t = np.cos(emb), np.sin(emb)

    def heads(w, b):
        return (x @ w + b).reshape(B, S, NH, D).transpose(0, 2, 1, 3)
    q, k, v = heads(Wq, bq), heads(Wk, bk), heads(Wv, bv)
    cos = cos_t[timestamps][:, None]
    sin = sin_t[timestamps][:, None]

    def rot(u):
        return np.concatenate((-u[..., D // 2:], u[..., :D // 2]), axis=-1)
    q = q * cos + rot(q) * sin
    k = k * cos + rot(k) * sin
    scores = np.einsum('bhqd,bhkd->bhqk', q, k) / np.sqrt(np.float32(D))
    ones = np.ones((T, T), np.float32)
    m = np.triu(ones, k=-CF).T * np.triu(ones, k=-CB)
    m = np.tile(m, (SP, SP))
    mask = (m[None, None] * attn_mask[:, None, None, :]) > 0
    scores = np.where(mask, scores, -1e9)
    scores -= scores.max(axis=-1, keepdims=True)
    e = np.exp(scores)
    attn = e / e.sum(axis=-1, keepdims=True)
    out = np.einsum('bhqk,bhkd->bhqd', attn, v)
    out = out.transpose(0, 2, 1, 3).reshape(B, S, H)
    return (out @ Wo + bo).astype(np.float32)


def kernel(x, attn_mask, timestamps, Wq, bq, Wk, bk, Wv, bv, Wo, bo,
           **_ignored):
    x = np.asarray(x, np.float32)
    attn_mask = np.asarray(attn_mask)
    timestamps = np.asarray(timestamps)
    Wq, Wk, Wv, Wo = (np.asarray(a, np.float32) for a in (Wq, Wk, Wv, Wo))
    bq, bk, bv, bo = (np.asarray(a, np.float32) for a in (bq, bk, bv, bo))

    # the device program bakes in the time-patch structure, an all-ones
    # attn_mask, and zero qkv biases; anything else takes the numpy path
    ts_tm = (timestamps.reshape(B, SP, T).swapaxes(1, 2).reshape(B, S))
    expect_ts = np.broadcast_to(
        np.repeat(np.arange(T, dtype=ts_tm.dtype), SP), (B, S))
    if (not np.array_equal(ts_tm, expect_ts)
            or not np.all(attn_mask == 1)
            or np.any(bq) or np.any(bk) or np.any(bv)):
        return _numpy_fallback(x, attn_mask, timestamps,
                               Wq, bq, Wk, bk, Wv, bv, Wo, bo)

    nc = _get_program()
    in_maps = _host_prep(x, Wq, Wk, Wv, Wo)

    res = bass_utils.run_bass_kernel_spmd(nc, in_maps,
                                          core_ids=list(range(N_CORES)))
    _CACHE["last_results"] = res

    out = np.empty((B, S, H), np.float32)
    for b in range(B):
        o = (res.results[2 * b]["outT"].astype(np.float32) +
             res.results[2 * b + 1]["outT"].astype(np.float32))
        o_tm = o.T + bo[None, :]                        # [2000, 512]
        out[b] = (o_tm.reshape(T, SP, H)
                      .swapaxes(0, 1)
                      .reshape(S, H))
    return out


# revision 12
# speedup vs baseline: 1.1556x; 1.0272x over previous
"""Banded (sparse) attention encoder block on 8 Trainium2 NeuronCores.

Problem: nn_NeuralEncoder (B=4, S=2000=100 time patches x 20 space patches,
H=512, 8 heads, D=64, RoPE over time-patch timestamps, banded attention
|t_q - t_k| <= 4 tiled over space patches).

Sharding: 8 cores = 4 batches x 2 head-groups (4 heads each).
Host prep: permute tokens to time-major order (u = t*SP + sp) so the banded
mask becomes a contiguous band of keys; transpose x to xT [H, S]; weights and
tables pre-packed into a few fully-contiguous DMA blobs; the RoPE sin table
carries the rotate-half signs (the rotate permutation itself is unsigned).

Device (one SPMD Bass program, all matmuls bf16 with fp32 PSUM):
  - 8 big input DMAs split across both HWDGE rings, first-needed first
  - q/k projection: 4 matmuls -> psum; DVE multiplies psum by per-patch
    cos/sin tables (broadcast APs); rotate-half via an unsigned PE
    permutation matmul; DVE adds the two halves into qT/kT
  - attn_mask==1 fast path: no mask scaling; the softmax denominator column
    of v is a one-time memset
  - per chunk j (100 keys): v projection (issued 2 chunks early to keep the
    PE stream dense; psum -> bf16 on ACT); 4 heads' scoresT into one 4-bank
    psum tile, ONE exp activation evacuates all of them; band masking only
    multiplies the two staircase side-blocks (GPSIMD) - the center is always
    valid
  - AV strip [q, head, d] accumulated over <=3 chunks with a ones column as
    denominator; reciprocal+normalize on DVE; PE transposes rebuild ctxT;
    output projection + DMA interleaved one 128-row column block per chunk
Host epilogue: sum the two head-group partials, add bo, transpose, un-permute
back to space-major order. Falls back to a numpy reference path if the inputs
don't match the expected mask/timestamp structure.
"""

import numpy as np
import ml_dtypes
from contextlib import ExitStack

import concourse.tile as tile
from concourse import bacc, mybir
from concourse import bass_utils

F32 = mybir.dt.float32
BF16 = mybir.dt.bfloat16

# Static problem configuration (hardcoded, matches the reference).
B, T, SP = 4, 100, 20
S = T * SP                  # 2000
H, NH, D = 512, 8, 64
CF = CB = 4
G = 2                       # head groups (tensor-parallel factor)
HPC = NH // G               # heads per core = 4
HG = HPC * D                # 256 hidden per group
VP = 95                     # padded per-head v width (pad cols keep the PE
                            # array duty above the HAM clock-gate threshold)
ROPE_BASE = 10000.0
N_CORES = 8

PPC = 5                     # time patches per key chunk
CK = PPC * SP               # 100 keys per chunk
NCH = T // PPC              # 20 key chunks / query strips
SC = 500                    # free-dim chunk for [128, 500] psum tiles
NSC = S // SC               # 4
MW = 280                    # max scoresT query-window width

_CACHE = {}


def _qwin(j):
    """Token range of the query window covered by scoresT of key chunk j."""
    lo = max(0, PPC * j - PPC) * SP
    hi = min(T, PPC * j + PPC + CF) * SP
    return lo, hi


def _build_program():
    nc = bacc.Bacc("TRN2", target_bir_lowering=False, debug=False,
                   num_devices=N_CORES)

    xT = nc.dram_tensor("xT", [H, S], BF16, kind="ExternalInput").ap()
    wqk = nc.dram_tensor("wqk", [128, 8, HG], BF16,
                         kind="ExternalInput").ap()
    w2 = nc.dram_tensor("w2", [128, 2048], BF16, kind="ExternalInput").ap()
    csb = nc.dram_tensor("csb", [128, 328], BF16, kind="ExternalInput").ap()
    mblob = nc.dram_tensor("mblob", [CK, 280], BF16,
                           kind="ExternalInput").ap()
    outT = nc.dram_tensor("outT", [H, S], BF16, kind="ExternalOutput").ap()

    with ExitStack() as ctx:
        tc = ctx.enter_context(tile.TileContext(nc))
        consts = ctx.enter_context(tc.tile_pool(name="consts", bufs=1))
        work = ctx.enter_context(tc.tile_pool(name="work", bufs=48))
        psum = ctx.enter_context(tc.tile_pool(name="psum", bufs=8,
                                              space="PSUM"))

        # ---- persistent tiles (blob views keep the DMA count tiny) ----
        xt = [consts.tile([128, S], BF16, tag=f"xt{kc}", name=f"xt{kc}")
              for kc in range(4)]
        wqk_sb = consts.tile([128, 8, HG], BF16, tag="wqk")
        w2_sb = consts.tile([128, 2048], BF16, tag="w2")
        csb_sb = consts.tile([128, 328], BF16, tag="csb")
        mb_sb = consts.tile([CK, 280], BF16, tag="mb")
        wq_sb = wqk_sb[:, 0:4]
        wk_sb = wqk_sb[:, 4:8]
        wv_sb = w2_sb[:, 0:1024].rearrange("p (c m) -> p c m", m=HG)
        wo_sb = w2_sb[:, 1024:2048].rearrange("p (c m) -> p c m", m=H)
        cos_sb = csb_sb[:, 0:100]
        sin_sb = csb_sb[:, 100:200]
        p_sb = csb_sb[:, 200:328]
        mL_sb = mb_sb[:, 0:100]
        mR_sb = mb_sb[:, 100:180]
        id_sb = mb_sb[:, 180:280]

        qT = [consts.tile([128, S], BF16, tag=f"qT{hp}", name=f"qT{hp}")
              for hp in range(2)]
        kT = [consts.tile([128, S], BF16, tag=f"kT{hp}", name=f"kT{hp}")
              for hp in range(2)]
        ctx_all = consts.tile([128, 2, S], BF16, tag="ctx", name="ctx")
        v_all = consts.tile([CK, NCH, HPC, VP], BF16, tag="v", name="v")

        # ---- input DMAs: first-needed first, split across both rings ----
        nc.sync.dma_start(out=wqk_sb, in_=wqk)
        nc.sync.dma_start(out=xt[0][:, 0:1000], in_=xT[0:128, 0:1000])
        nc.sync.dma_start(out=xt[1][:, 0:1000], in_=xT[128:256, 0:1000])
        nc.sync.dma_start(out=w2_sb, in_=w2)
        nc.sync.dma_start(out=xt[0][:, 1000:S], in_=xT[0:128, 1000:S])
        nc.sync.dma_start(out=xt[1][:, 1000:S], in_=xT[128:256, 1000:S])
        nc.scalar.dma_start(out=xt[2][:, 0:1000], in_=xT[256:384, 0:1000])
        nc.scalar.dma_start(out=xt[3][:, 0:1000], in_=xT[384:512, 0:1000])
        nc.scalar.dma_start(out=csb_sb, in_=csb)
        nc.scalar.dma_start(out=mb_sb, in_=mblob)
        nc.scalar.dma_start(out=xt[2][:, 1000:S], in_=xT[256:384, 1000:S])
        nc.scalar.dma_start(out=xt[3][:, 1000:S], in_=xT[384:512, 1000:S])

        # softmax denominator column of v (attn_mask == 1)
        nc.gpsimd.memset(v_all[:, :, :, D], 1.0)

        # ---- q/k projections + RoPE ----
        def qk_proj(w_sb, dst, hp, sc):
            cols = slice(SC * sc, SC * (sc + 1))
            p0 = 25 * sc
            ps = psum.tile([128, SC], F32, tag="pp", bufs=2)
            for kc in range(4):
                nc.tensor.matmul(
                    ps,
                    lhsT=w_sb[:, kc, 128 * hp:128 * (hp + 1)],
                    rhs=xt[kc][:, cols],
                    start=(kc == 0), stop=(kc == 3),
                )
            psv = ps.rearrange("p (a b) -> p a b", b=SP)
            # t1 = ps * cos  -> straight into dst
            nc.vector.tensor_mul(
                out=dst[:, cols].rearrange("p (a b) -> p a b", b=SP),
                in0=psv,
                in1=cos_sb[:, p0:p0 + 25].unsqueeze(2)
                    .broadcast_to([128, 25, SP]))
            # t2 = ps * (sign-folded sin); unsigned rotate-half on the PE
            t2p = work.tile([128, SC], BF16, tag="t2p", bufs=3)
            nc.vector.tensor_mul(
                out=t2p.rearrange("p (a b) -> p a b", b=SP),
                in0=psv,
                in1=sin_sb[:, p0:p0 + 25].unsqueeze(2)
                    .broadcast_to([128, 25, SP]))
            psr = psum.tile([128, SC], F32, tag="pp", bufs=2)
            nc.tensor.matmul(psr, lhsT=p_sb, rhs=t2p, start=True, stop=True)
            nc.vector.tensor_add(out=dst[:, cols], in0=dst[:, cols],
                                 in1=psr)

        # ---- v projection (psum shares the [128, 500] "pp" ring) ----
        def v_proj(vc):
            rows = slice(CK * vc, CK * (vc + 1))
            ps = psum.tile([128, SC], F32, tag="pp", bufs=2)
            for kc in range(4):
                nc.tensor.matmul(
                    ps[0:CK, 0:HG],
                    lhsT=xt[kc][:, rows],
                    rhs=wv_sb[:, kc, :],
                    start=(kc == 0), stop=(kc == 3),
                )
            nc.scalar.copy(
                out=v_all[:, vc, :, 0:D],
                in_=ps[0:CK, 0:HG].rearrange("p (h e) -> p h e", e=D))

        # ---- attention ----
        exp_t = {}
        cs_t = {}

        def scores_chunk(j):
            qlo, qhi = _qwin(j)
            w = qhi - qlo
            et = work.tile([CK, HPC, MW], BF16, tag="et4", bufs=5)
            # all 4 heads' scores into one 4-bank psum tile; ONE exp
            ps = psum.tile([CK, HPC, 512], F32, tag="pss", bufs=1)
            for h in range(HPC):
                hp, hb = h // 2, 64 * (h % 2)
                nc.tensor.matmul(
                    ps[:, h, :w],
                    lhsT=kT[hp][hb:hb + 64, CK * j:CK * (j + 1)],
                    rhs=qT[hp][hb:hb + 64, qlo:qhi],
                    start=True, stop=True,
                )
            nc.scalar.activation(out=et[:, :, :w], in_=ps[:, :, :w],
                                 func=mybir.ActivationFunctionType.Exp,
                                 scale=0.125)
            # band masking: only the staircase side-blocks need a multiply
            if j > 0:
                nc.gpsimd.tensor_mul(
                    out=et[:, :, 0:CK], in0=et[:, :, 0:CK],
                    in1=mL_sb.unsqueeze(1).broadcast_to([CK, HPC, CK]))
            rlo = 100 if j == 0 else (None if j == NCH - 1 else 200)
            if rlo is not None:
                nc.gpsimd.tensor_mul(
                    out=et[:, :, rlo:rlo + 80], in0=et[:, :, rlo:rlo + 80],
                    in1=mR_sb.unsqueeze(1).broadcast_to([CK, HPC, 80]))
            exp_t[j] = et

        def av_mm(i):
            # chunk i first: it covers the strip fully (start=True sets
            # has_written; the left neighbor accumulates on partitions 0:80)
            chunks = [c for c in (i, i - 1, i + 1) if 0 <= c < NCH]
            ps = psum.tile([CK, HPC, VP], F32, tag="pav", bufs=1)
            for h in range(HPC):
                for n, j in enumerate(chunks):
                    qlo, qhi = _qwin(j)
                    lo_g, hi_g = max(CK * i, qlo), min(CK * i + CK, qhi)
                    nc.tensor.matmul(
                        ps[0:hi_g - lo_g, h, :],
                        lhsT=exp_t[j][:, h, lo_g - qlo:hi_g - qlo],
                        rhs=v_all[:, j, h, :],
                        start=(n == 0), stop=(n == len(chunks) - 1),
                    )
            rcp = work.tile([CK, HPC], F32, tag="rcp", bufs=3)
            nc.vector.reciprocal(out=rcp, in_=ps[:, :, D])
            cs = work.tile([CK, HPC, D], BF16, tag="cs", bufs=3)
            nc.vector.tensor_mul(
                out=cs, in0=ps[:, :, 0:D],
                in1=rcp.unsqueeze(2).broadcast_to([CK, HPC, D]))
            cs_t[i] = cs

        def av_tr(i):
            csf = cs_t.pop(i).rearrange("p h e -> p (h e)")
            pt = psum.tile([128, 2, CK], BF16, tag="ptr", bufs=1)
            for hp in range(2):
                nc.tensor.transpose(pt[:, hp, :],
                                    csf[:, 128 * hp:128 * (hp + 1)], id_sb)
            nc.vector.tensor_copy(out=ctx_all[:, :, CK * i:CK * (i + 1)],
                                  in_=pt)

        # ---- output projection, one 128-row column block at a time ----
        def out_oc(c, oc, lo=0, hi=SC):
            w = hi - lo
            cols = slice(SC * c + lo, SC * c + hi)
            ps = psum.tile([128, SC], F32, tag="pp", bufs=2)
            for hp in range(2):
                nc.tensor.matmul(
                    ps[:, :w],
                    lhsT=wo_sb[:, hp, 128 * oc:128 * (oc + 1)],
                    rhs=ctx_all[:, hp, cols],
                    start=(hp == 0), stop=(hp == 1),
                )
            ost = work.tile([128, SC], BF16, tag="ost", bufs=3)
            nc.scalar.copy(out=ost[:, :w], in_=ps[:, :w])
            nc.sync.dma_start(out=outT[128 * oc:128 * (oc + 1), cols],
                              in_=ost[:, :w])

        # ---- software-pipelined main loop ----
        def qk_call(sc, m):
            hp = m % 2
            if m < 2:
                qk_proj(wq_sb, qT[hp], hp, sc)
            else:
                qk_proj(wk_sb, kT[hp], hp, sc)

        ranges = [0, 4, 9, 14, NCH]
        for m in range(4):
            qk_call(0, m)
        # v is independent of RoPE; run it 2 chunks ahead to keep the PE
        # stream dense from the start
        v_proj(0)
        v_proj(1)
        for sc in range(NSC):
            for idx, j in enumerate(range(ranges[sc], ranges[sc + 1])):
                if sc + 1 < NSC and idx < 4:
                    qk_call(sc + 1, idx)
                if j + 2 < NCH:
                    v_proj(j + 2)
                scores_chunk(j)
                if j >= 2:
                    av_tr(j - 2)
                if j >= 1:
                    av_mm(j - 1)
                if j >= 6 and (j - 6) % 5 < 4 and (j - 6) // 5 < 3:
                    out_oc((j - 6) // 5, (j - 6) % 5)
        # epilogue: drain the pipeline
        av_mm(NCH - 1)
        av_tr(NCH - 2)
        av_tr(NCH - 1)
        for oc in range(4):
            out_oc(3, oc)

    nc.finalize()   # Bacc register allocation + DCE before serialization
    return nc


def _get_program():
    if "nc" not in _CACHE:
        _CACHE["nc"] = _build_program()
    return _CACHE["nc"]


def _host_prep(x, Wq, Wk, Wv, Wo):
    """Build the 8 per-core input maps."""
    bf16 = ml_dtypes.bfloat16

    def to_tm(a):
        # [B, S, ...] space-major -> time-major (u = t*SP + sp)
        return (a.reshape(B, SP, T, *a.shape[2:])
                 .swapaxes(1, 2)
                 .reshape(B, S, *a.shape[2:]))

    x_tm = to_tm(np.ascontiguousarray(x))

    # RoPE tables, per time patch; sin carries the rotate-half signs
    inv_freq = 1.0 / (ROPE_BASE ** (np.arange(0, D, 2, dtype=np.float32) / D))
    tt = np.arange(T, dtype=np.float32)
    freqs = tt[:, None] * inv_freq[None, :]
    emb = np.concatenate([freqs, freqs], axis=-1)      # [T, D]
    cos_t = np.cos(emb).astype(np.float32).T           # [64, T]
    sin_t = np.sin(emb).astype(np.float32).T
    sinS = sin_t.copy()
    sinS[D // 2:] *= -1.0                              # sign fold for rotate

    # unsigned rotate-half permutation (sinS already carries the signs)
    p = np.zeros((128, 128), np.float32)
    for blk in (0, 64):
        for d in range(32):
            p[blk + d + 32, blk + d] = 1.0
            p[blk + d, blk + d + 32] = 1.0
    csb = np.concatenate([np.vstack([cos_t, cos_t]),
                          np.vstack([sinS, sinS]), p], axis=1)  # [128, 328]

    # staircase band masks for the side-blocks + transpose identity
    kr = np.arange(CK)[:, None] // SP
    cl = np.arange(CK)[None, :] // SP
    m2L = (cl >= kr + 1).astype(np.float32)            # [100, 100]
    cr = np.arange(80)[None, :] // SP
    m2R = (cr <= kr - 1).astype(np.float32)            # [100, 80]
    mblob = np.concatenate([m2L, m2R, np.eye(CK, dtype=np.float32)],
                           axis=1)                     # [100, 280]

    def pack4(w):       # [512, M] -> [128, 4, M]
        return np.ascontiguousarray(
            w.reshape(4, 128, w.shape[1]).transpose(1, 0, 2))

    in_maps = []
    for c in range(N_CORES):
        b, g = c // 2, c % 2
        hcols = slice(HG * g, HG * (g + 1))
        wqk = np.concatenate([pack4(Wq[:, hcols]), pack4(Wk[:, hcols])],
                             axis=1)                   # [128, 8, 256]
        w2 = np.concatenate(
            [pack4(Wv[:, hcols]).reshape(128, 1024),
             np.ascontiguousarray(
                 Wo[hcols, :].reshape(2, 128, H).transpose(1, 0, 2)
             ).reshape(128, 1024)], axis=1)            # [128, 2048]
        in_maps.append({
            "xT": np.ascontiguousarray(x_tm[b].T).astype(bf16),
            "wqk": wqk.astype(bf16),
            "w2": w2.astype(bf16),
            "csb": csb.astype(bf16),
            "mblob": mblob.astype(bf16),
        })
    return in_maps


def _numpy_fallback(x, attn_mask, timestamps, Wq, bq, Wk, bk, Wv, bv, Wo, bo):
    """Reference-equivalent numpy path for unexpected input structure."""
    inv_freq = 1.0 / (ROPE_BASE ** (np.arange(0, D, 2, dtype=np.float32) / D))
    tt = np.arange(T, dtype=np.float32)
    emb = np.concatenate([tt[:, None] * inv_freq[None, :]] * 2, axis=-1)
    cos_t, sin_t = np.cos(emb), np.sin(emb)

    def heads(w, b):
        return (x @ w + b).reshape(B, S, NH, D).transpose(0, 2, 1, 3)
    q, k, v = heads(Wq, bq), heads(Wk, bk), heads(Wv, bv)
    cos = cos_t[timestamps][:, None]
    sin = sin_t[timestamps][:, None]

    def rot(u):
        return np.concatenate((-u[..., D // 2:], u[..., :D // 2]), axis=-1)
    q = q * cos + rot(q) * sin
    k = k * cos + rot(k) * sin
    scores = np.einsum('bhqd,bhkd->bhqk', q, k) / np.sqrt(np.float32(D))
    ones = np.ones((T, T), np.float32)
    m = np.triu(ones, k=-CF).T * np.triu(ones, k=-CB)
    m = np.tile(m, (SP, SP))
    mask = (m[None, None] * attn_mask[:, None, None, :]) > 0
    scores = np.where(mask, scores, -1e9)
    scores -= scores.max(axis=-1, keepdims=True)
    e = np.exp(scores)
    attn = e / e.sum(axis=-1, keepdims=True)
    out = np.einsum('bhqk,bhkd->bhqd', attn, v)
    out = out.transpose(0, 2, 1, 3).reshape(B, S, H)
    return (out @ Wo + bo).astype(np.float32)


def kernel(x, attn_mask, timestamps, Wq, bq, Wk, bk, Wv, bv, Wo, bo,
           **_ignored):
    x = np.asarray(x, np.float32)
    attn_mask = np.asarray(attn_mask)
    timestamps = np.asarray(timestamps)
    Wq, Wk, Wv, Wo = (np.asarray(a, np.float32) for a in (Wq, Wk, Wv, Wo))
    bq, bk, bv, bo = (np.asarray(a, np.float32) for a in (bq, bk, bv, bo))

    # the device program bakes in the time-patch structure, an all-ones
    # attn_mask, and zero qkv biases; anything else takes the numpy path
    ts_tm = (timestamps.reshape(B, SP, T).swapaxes(1, 2).reshape(B, S))
    expect_ts = np.broadcast_to(
        np.repeat(np.arange(T, dtype=ts_tm.dtype), SP), (B, S))
    if (not np.array_equal(ts_tm, expect_ts)
            or not np.all(attn_mask == 1)
            or np.any(bq) or np.any(bk) or np.any(bv)):
        return _numpy_fallback(x, attn_mask, timestamps,
                               Wq, bq, Wk, bk, Wv, bv, Wo, bo)

    nc = _get_program()
    in_maps = _host_prep(x, Wq, Wk, Wv, Wo)

    res = bass_utils.run_bass_kernel_spmd(nc, in_maps,
                                          core_ids=list(range(N_CORES)))
    _CACHE["last_results"] = res

    out = np.empty((B, S, H), np.float32)
    for b in range(B):
        o = (res.results[2 * b]["outT"].astype(np.float32) +
             res.results[2 * b + 1]["outT"].astype(np.float32))
        o_tm = o.T + bo[None, :]                        # [2000, 512]
        out[b] = (o_tm.reshape(T, SP, H)
                      .swapaxes(0, 1)
                      .reshape(S, H))
    return out
